# revision 47
# baseline (speedup 1.0000x reference)
"""Self-contained Trainium2 Bass kernel for nn_MultiHeadAttention_80599356276988.

Strategy: tensor-parallel over heads (2 heads/core x 8 cores). Software-
pipelined phases: QKV projection units (phase A) are interleaved into the
attention chains (phase B) so the scalar engine (softmax exp, the critical
resource) starts ~17us in and never waits; head-split AllToAll so the first
collective overlaps the second head's compute; per-core output projection of
512 rows. All operands bf16 (host-side cast): halves HBM traffic, every
matmul runs at 1 cycle/row.
"""
import sys

sys.path.insert(0, "/opt/trn_rl_repo")
import numpy as np
from contextlib import ExitStack

import concourse.bass as bass
import concourse.mybir as mybir
import concourse.tile as tile
from concourse import bacc
from concourse.bass_utils import run_bass_kernel_spmd

F32 = mybir.dt.float32
BF16 = mybir.dt.bfloat16
EXP = mybir.ActivationFunctionType.Exp

B, T, C = 2, 2048, 1024
H, D = 16, 64
NCORES = 8
HPC = H // NCORES        # heads per core = 2
DPC = HPC * D            # dims per core = 128
N = B * T                # 4096 flat rows
RPC = N // NCORES        # output rows per core = 512
SCALE = float(C) ** -0.5
TC4 = T // 512           # 4 t-chunks of 512 per batch
NT8 = N // 512           # 8 flat t-chunks of 512

_CACHE = {}


def build_nc():
    nc = bacc.Bacc(num_devices=NCORES)

    # X^T per contraction chunk: XTC[cc] = [128, 4096] bf16 (rows contiguous)
    XTC = nc.dram_tensor("xtc", [8, 128, N], BF16, kind="ExternalInput")
    WQ3 = nc.dram_tensor("wq3", [128, C], BF16, kind="ExternalInput")
    WK3 = nc.dram_tensor("wk3", [128, C], BF16, kind="ExternalInput")
    WV3 = nc.dram_tensor("wv3", [128, C], BF16, kind="ExternalInput")
    WPT = nc.dram_tensor("wpt8", [C, C], BF16, kind="ExternalInput")
    BIAS = nc.dram_tensor("bias", [1, C], F32, kind="ExternalInput")
    CMASK = nc.dram_tensor("cmask", [4, 128, 512], BF16, kind="ExternalInput")
    IDENT = nc.dram_tensor("ident", [128, 128], BF16, kind="ExternalInput")
    OUT = nc.dram_tensor("out", [RPC, C], F32, kind="ExternalOutput")

    a2a_in = [nc.dram_tensor(f"a2a_in{h}", [NCORES, 64, RPC], BF16)
              for h in range(HPC)]
    a2a_out = [nc.dram_tensor(f"a2a_out{h}", [NCORES, 64, RPC], BF16)
               for h in range(HPC)]

    with tile.TileContext(nc) as tc, ExitStack() as ctx:
        consts = ctx.enter_context(tc.tile_pool(name="consts", bufs=1))
        qkv = ctx.enter_context(tc.tile_pool(name="qkv", bufs=1))

        wq_sb = consts.tile([128, C], BF16, tag="wq")
        wk_sb = consts.tile([128, C], BF16, tag="wk")
        wv_sb = consts.tile([128, C], BF16, tag="wv")
        ident_sb = consts.tile([128, 128], BF16, tag="ident")
        wpt_sb = [consts.tile([128, C], BF16, tag=f"wpt{j}", name=f"wpt{j}")
                  for j in range(8)]
        cmask_sb = consts.tile([128, 4 * 512], BF16, tag="cmask")
        bias_sb = consts.tile([128, C], F32, tag="bias")

        QT = qkv.tile([128, N], BF16, tag="QT")
        KTz = qkv.tile([128, 2 * N], BF16, tag="KTz")
        VS = qkv.tile([128, 4 * 1040], BF16, tag="VS")
        XS = [qkv.tile([128, N], BF16, tag=f"xs{cc}", name=f"xs{cc}")
              for cc in range(8)]

        # ---- DMA issue order ----
        # Sync queue: X only (t8=0 as 512-col slices for a fast first matmul,
        # the rest as 1024-col slices for 2KB DMA lines).
        for lo, hi in ((0, 1024), (1024, 2048), (2048, 3072), (3072, 4096)):
            for cc in range(8):
                nc.sync.dma_start(XS[cc][:, lo:hi], XTC[cc][:, lo:hi])
        # Scalar (hwdge) queue: projection weights + transpose identity, in
        # parallel with X, landing before the first A chain completes.
        nc.scalar.dma_start(wq_sb[:], WQ3[:])
        nc.scalar.dma_start(wk_sb[:], WK3[:])
        nc.scalar.dma_start(wv_sb[:], WV3[:])
        nc.scalar.dma_start(ident_sb[:], IDENT[:])
        for k in range(4):
            nc.scalar.dma_start(cmask_sb[:, k * 512:(k + 1) * 512], CMASK[k])
        # Phase-D constants ride the sync queue behind X (done by ~40us).
        nc.sync.dma_start(bias_sb[:], BIAS[0:1, :].to_broadcast((128, C)))
        for j in range(8):
            nc.sync.dma_start(wpt_sb[j][:], WPT[j * 128:(j + 1) * 128, :])
        # GpSimd: zero/one fills only (KTz pads first — scores need them).
        nc.gpsimd.memset(KTz[64:128, 0:N // 2], 0.0)
        nc.gpsimd.memset(KTz[0:64, N:N + N // 2], 0.0)
        nc.gpsimd.memset(KTz[64:128, N // 2:N], 0.0)
        nc.gpsimd.memset(KTz[0:64, N + N // 2:2 * N], 0.0)
        for b in range(B):
            for h in range(HPC):
                base = (b * HPC + h) * 1040
                nc.gpsimd.memset(VS[:, base + 64:base + 1040:65], 1.0)

        # ---- pools: B pools outer (live whole kernel), A pools inner ----
        phB = ctx.enter_context(tc.tile_pool(name="phB", bufs=2))
        psB_g = ctx.enter_context(tc.tile_pool(name="psBg", bufs=2, space="PSUM"))
        psB_av = ctx.enter_context(tc.tile_pool(name="psBav", bufs=1, space="PSUM"))

        a_stack = ExitStack()
        phA = a_stack.enter_context(tc.tile_pool(name="phA", bufs=1))
        psA = a_stack.enter_context(tc.tile_pool(name="psA", bufs=2, space="PSUM"))
        psT = a_stack.enter_context(tc.tile_pool(name="psT", bufs=1, space="PSUM"))
        VT = phA.tile([128, N], BF16, tag="VT")

        # ---------------- phase A units ----------------
        # phase A units as closures: each t8 chunk becomes three filler units
        # (Q/K/V chains interleaved over three psum tiles so consecutive
        # matmuls never serialize on the same accumulator).
        a_state = {}

        def a_qk(t8):
            # Q and K chains cc-interleaved across the two psA slots so
            # consecutive matmuls hit different accumulators
            cols = slice(t8 * 512, (t8 + 1) * 512)
            ppq = psA.tile([128, 512], F32, tag="pp", name="ppq")
            ppk = psA.tile([128, 512], F32, tag="pp", name="ppk")
            for cc in range(8):
                for pp, w_sb in ((ppq, wq_sb), (ppk, wk_sb)):
                    nc.tensor.matmul(
                        pp[:], w_sb[:, cc * 128:(cc + 1) * 128],
                        XS[cc][:, cols],
                        start=(cc == 0), stop=(cc == 7))
            nc.vector.tensor_copy(QT[:, cols], ppq[:])
            nc.vector.tensor_copy(KTz[0:64, cols], ppk[0:64, :])
            nc.vector.tensor_copy(
                KTz[64:128, N + t8 * 512:N + (t8 + 1) * 512],
                ppk[64:128, :])

        def a_v(t8):
            cols = slice(t8 * 512, (t8 + 1) * 512)
            ppv = psA.tile([128, 512], F32, tag="pp", name="ppv")
            for cc in range(8):
                nc.tensor.matmul(
                    ppv[:], wv_sb[:, cc * 128:(cc + 1) * 128],
                    XS[cc][:, cols],
                    start=(cc == 0), stop=(cc == 7))
            nc.vector.tensor_copy(VT[:, cols], ppv[:])

        def a_transposes(t8):
            b = t8 // 4
            ptr = psT.tile([128, 512], BF16, tag="ptr")
            for q in range(4):
                st = (t8 % 4) * 4 + q
                nc.tensor.transpose(
                    ptr[:, q * 128:(q + 1) * 128],
                    VT[:, b * T + st * 128:b * T + (st + 1) * 128],
                    ident_sb[:])
                for h in range(HPC):
                    base = (b * HPC + h) * 1040
                    nc.vector.tensor_copy(
                        VS[:, base + st * 65:base + st * 65 + 64],
                        ptr[:, q * 128 + h * 64:q * 128 + (h + 1) * 64])

        def emit_A(t8):
            a_qk(t8)
            a_v(t8)
            a_transposes(t8)

        # ---------------- phase B chains ----------------
        LOOKAHEAD = 4
        state = {}

        def av_half(item, half):
            b, h, t4, g, Pg = item
            vbase = (b * HPC + h) * 1040
            last = 4 * t4 + 3
            st = 2 * g + half
            if st == 0:
                state[(b, h, t4)] = psB_av.tile([65, 512], F32, tag="pav",
                                                name="pav")
            pav = state[(b, h, t4)]
            nc.tensor.matmul(
                pav[0:65, :],
                VS[:, vbase + st * 65:vbase + st * 65 + 65],
                Pg[:, half * 512:(half + 1) * 512],
                start=(st == 0), stop=(st == last))
            if st == last:
                act_sb = phB.tile([64, 512], BF16, tag="actsb", name="act_sb",
                                  bufs=8)
                nc.vector.tensor_copy(act_sb[:], pav[0:64, :])
                sums = phB.tile([1, 512], F32, tag="sums", name="sums", bufs=8)
                nc.vector.tensor_copy(sums[:], pav[64:65, :])
                rec = phB.tile([1, 512], F32, tag="rec", name="rec", bufs=8)
                nc.vector.reciprocal_approx_fast(out=rec[:], in_=sums[:])
                rb = phB.tile([64, 512], BF16, tag="rb", name="rb", bufs=8)
                nc.vector.tensor_copy(rb[0:1, :], rec[:])
                nc.gpsimd.partition_broadcast(rb[:], rb[0:1, :], channels=64)
                act_t = phB.tile([64, 512], BF16, tag="act", name="act_t",
                                 bufs=8)
                nc.vector.tensor_mul(act_t[:], act_sb[:], rb[:])
                nc.sync.dma_start(a2a_in[h][b * TC4 + t4], act_t[:])

        # A(0)'s V-chain and transposes ride the filler queue (only its Q/K
        # part gates the first scores); fillers pop at the TOP of each group
        # so every unit lands ahead of the chain that consumes it.
        fillers = [(0, 1), (0, 2)] + \
                  [(t8, u) for t8 in range(1, NT8) for u in range(3)]

        def emit_filler():
            if fillers:
                t8, u = fillers.pop(0)
                if u == 0:
                    a_qk(t8)
                elif u == 1:
                    a_v(t8)
                else:
                    a_transposes(t8)

        def emit_chain(b, h, t4):
            bcol = b * T
            qcol = bcol + t4 * 512
            pending = []
            for g in range(2 * t4 + 2):
                emit_filler()
                psg = psB_g.tile([128, 1024], F32, tag="psg", name="psg")
                av = pending.pop(0) if len(pending) > LOOKAHEAD else None
                for half in range(2):
                    st = 2 * g + half
                    nc.tensor.matmul(
                        psg[:, half * 512:(half + 1) * 512],
                        KTz[:, h * N + bcol + st * 128:
                            h * N + bcol + (st + 1) * 128],
                        QT[:, qcol:qcol + 512],
                        start=True, stop=True)
                    if av is not None:
                        av_half(av, half)
                Pg = phB.tile([128, 1024], BF16, tag="P", name="Pg",
                              bufs=LOOKAHEAD + 2)
                nc.scalar.activation(Pg[:], psg[:], EXP, scale=SCALE)
                for half in range(2):
                    st = 2 * g + half
                    if st >= 4 * t4:
                        k = st - 4 * t4
                        sl = Pg[:, half * 512:(half + 1) * 512]
                        nc.vector.tensor_mul(
                            sl, sl, cmask_sb[:, k * 512:(k + 1) * 512])
                pending.append((b, h, t4, g, Pg))
            for item in pending:
                av_half(item, 0)
                av_half(item, 1)

        a_qk(0)

        a_closed = False
        rv = []
        for h in range(HPC):
            for b in range(B):
                for t4 in range(TC4):
                    emit_chain(b, h, t4)
                    if not fillers and not a_closed:
                        a_stack.close()
                        a_closed = True
            nc.gpsimd.collective_compute(
                "AllToAll", mybir.AluOpType.bypass,
                replica_groups=[list(range(NCORES))],
                ins=[a2a_in[h][:]], outs=[a2a_out[h][:]])
            if h == 0:
                # preload the h0 halves of rv during h1 compute
                rvp = ctx.enter_context(tc.tile_pool(name="rvp", bufs=1))
                for j in range(8):
                    r = rvp.tile([128, 512], BF16, tag=f"rv{j}", name=f"rv{j}")
                    nc.sync.dma_start(r[0:64, :], a2a_out[0][j])
                    rv.append(r)
            else:
                for j in range(8):
                    eng = nc.sync if j % 2 == 0 else nc.scalar
                    eng.dma_start(rv[j][64:128, :], a2a_out[1][j])

        # ---------------- Phase D: output projection ----------------
        with tc.tile_pool(name="phD", bufs=2) as phD, \
             tc.tile_pool(name="psD", bufs=2, space="PSUM") as psD:
            for mt in range(4):
                po = [psD.tile([128, 512], F32, tag=f"po{oc}", name=f"po{oc}",
                               bufs=1)
                      for oc in range(2)]
                # j-outer, oc-inner: both output halves share the stationary
                # and alternate accumulators (no psum serialization)
                for j in range(8):
                    for oc in range(2):
                        nc.tensor.matmul(
                            po[oc][:],
                            rv[j][:, mt * 128:(mt + 1) * 128],
                            wpt_sb[j][:, oc * 512:(oc + 1) * 512],
                            start=(j == 0), stop=(j == 7))
                for oc in range(2):
                    ot = phD.tile([128, 512], F32, tag=f"ot{oc}",
                                  name=f"ot{oc}", bufs=1)
                    nc.vector.tensor_add(
                        ot[:], po[oc][:], bias_sb[:, oc * 512:(oc + 1) * 512])
                    nc.sync.dma_start(
                        OUT[mt * 128:(mt + 1) * 128, oc * 512:(oc + 1) * 512],
                        ot[:])

    nc.compile()
    return nc


def prep_in_maps(X, Wq, Wk, Wv, Wp, bp):
    bf16 = mybir.dt.np(BF16)
    X = np.asarray(X, dtype=np.float32)
    Wq = np.asarray(Wq, dtype=np.float32)
    Wk = np.asarray(Wk, dtype=np.float32)
    Wv = np.asarray(Wv, dtype=np.float32)
    Wp = np.asarray(Wp, dtype=np.float32)
    bp = np.asarray(bp, dtype=np.float32)

    XT = X.reshape(N, C).T                                   # [C, N]
    XTC = np.ascontiguousarray(XT.reshape(8, 128, N)).astype(bf16)
    WPT = np.ascontiguousarray(Wp.T).astype(bf16)
    bias = np.ascontiguousarray(bp.reshape(1, C))

    cmask = np.zeros((4, 128, 512), dtype=np.float32)
    for k in range(4):
        p = np.arange(128)[:, None]
        f = np.arange(512)[None, :]
        cmask[k] = (128 * k + p <= f).astype(np.float32)
    cmask = cmask.astype(bf16)
    ident = np.eye(128, dtype=np.float32).astype(bf16)

    def w3(Wfull, i):
        Wc = Wfull[HPC * i:HPC * i + HPC].reshape(DPC, C)
        WT = np.ascontiguousarray(Wc.T)
        return np.ascontiguousarray(
            WT.reshape(8, 128, DPC).transpose(1, 0, 2).reshape(128, C)
        ).astype(bf16)

    in_maps = []
    for i in range(NCORES):
        in_maps.append({
            "xtc": XTC,
            "wq3": w3(Wq, i),
            "wk3": w3(Wk, i),
            "wv3": w3(Wv, i),
            "wpt8": WPT,
            "bias": bias,
            "cmask": cmask,
            "ident": ident,
        })
    return in_maps


def run(inputs, trace=False, trace_kwargs=None):
    if "nc" not in _CACHE:
        _CACHE["nc"] = build_nc()
    nc = _CACHE["nc"]
    in_maps = prep_in_maps(**inputs)
    res = run_bass_kernel_spmd(
        nc, in_maps, list(range(NCORES)), trace=trace,
        **(trace_kwargs or {}))
    out = np.concatenate([res.results[i]["out"] for i in range(NCORES)], axis=0)
    return out.reshape(B, T, C), res


def kernel(**inputs) -> np.ndarray:
    out, _ = run(inputs, trace=False)
    return out
